# revision 5
# baseline (speedup 1.0000x reference)
"""Trainium2 Bass kernel for a 4-layer post-norm transformer encoder stack.

Sharding: data-parallel over batch (B=8) across 8 NeuronCores; no collectives.
Per-core layout: feature-major activations [128, nj, 1024] bf16; feature
d = j*128 + p.  Matmuls are bf16 x bf16 with fp32 PSUM accumulation; LN
second moments go through f32r tiles to keep the variance path accurate.

Weights are packed host-side into the exact SBUF consumption layout and
concatenated into a single DRAM tensor, so every weight load is one
contiguous-row 2D DMA (8KB per partition row) streamed once per execution on
the SP hardware DGE queue; activations/constants/output ride the
Activation-engine DGE queue.  Only 3 input tensors are bound per core
(weights, consts, x) -- per-dispatch overhead scales with tensor count.
"""

import numpy as np
from contextlib import ExitStack

import concourse.bass as bass
from concourse import bacc
import concourse.tile as tile
from concourse import mybir
from concourse.bass_utils import run_bass_kernel_spmd
import ml_dtypes

F32 = mybir.dt.float32
F32R = mybir.dt.float32r
BF16 = mybir.dt.bfloat16
BF = ml_dtypes.bfloat16
AF = mybir.ActivationFunctionType
ALU = mybir.AluOpType

S, B, E, D, H, FF, L = 1024, 8, 768, 1024, 8, 2048, 4
DH = D // H
T = S          # tokens per core
P = 128
CH = 512       # free-dim chunk (one PSUM bank of fp32)
NCH = T // CH  # 2
EPS = 1e-5
N_CORES = 8

# qkv consumption order: k_h, q_h interleaved so attention head h can start
# as soon as its rows land. Row index in qkvT = original do (q: 0..7, k: 8..15).
QKV_ORDER = []
for _h in range(H):
    QKV_ORDER += [8 + _h, _h]


def _const_layout():
    off = {}
    c = 0

    def add(name, n):
        nonlocal c
        off[name] = c
        c += n

    add("enc_b", 8), add("enc_ng", 8), add("enc_gb", 8)
    for l in range(L):
        add(f"qk_b_{l}", 16)       # consumption (QKV_ORDER) order
        add(f"out_b_{l}", 8)
        add(f"ln1_ng_{l}", 8), add(f"ln1_b_{l}", 8)
        add(f"ff1_b_{l}", 16)
        add(f"ff2_b_{l}", 8)
        add(f"ln2_ng_{l}", 8), add(f"ln2_b_{l}", 8)
        add(f"vb_{l}", 8)          # V bias, per-head dh-partition columns
    add("dec_b1", 4), add("dec_ng", 4), add("dec_gb", 4), add("dec_b2", 6)
    return off, c


COFF, CF = _const_layout()

# dense geometry: name -> (kj_n, n_do, slab_dos)
DENSES = {
    "enc": (6, 8, 4),
    "qk": (8, 16, 4),
    "out": (8, 8, 4),
    "ff1": (8, 16, 4),
    "ff2": (16, 8, 2),
    "dec1": (8, 4, 4),
    "dec2": (4, 6, 6),
}


def _wcols(key):
    kj_n, n_do, _ = DENSES[key]
    return n_do * kj_n * P


def _weight_layout():
    """Column offsets of every block inside the single packed weight tensor."""
    off = {}
    c = 0

    def add(name, n):
        nonlocal c
        off[name] = c
        c += n

    add("ones", 3 * P)
    add("enc", _wcols("enc"))
    for l in range(L):
        add(f"qk_{l}", _wcols("qk"))
        add(f"v_{l}", 2 * 8 * CH)
        add(f"out_{l}", _wcols("out"))
        add(f"ff1_{l}", _wcols("ff1"))
        add(f"ff2_{l}", _wcols("ff2"))
    add("dec1", _wcols("dec1"))
    add("dec2", _wcols("dec2"))
    return off, c


WOFF, WCOLS = _weight_layout()


# ----------------------------------------------------------------- builder --

def _build():
    nc = bacc.Bacc("TRN2", target_bir_lowering=False, debug=False,
                   num_devices=N_CORES)

    def inp(name, shape, dt=BF16):
        return nc.declare_dram_parameter(name, list(shape), dt, isOutput=False)

    xT_d = inp("xT", (P, 6 * T))
    consts_d = inp("consts", (P, CF), F32)
    wts_d = inp("wts", (P, WCOLS))

    out_d = nc.declare_dram_parameter("out", [E, T], F32, isOutput=True)

    with tile.TileContext(nc) as tc, ExitStack() as ctx:
        act = ctx.enter_context(tc.tile_pool(name="act", bufs=3))
        big = ctx.enter_context(tc.tile_pool(name="big", bufs=1))
        wp = ctx.enter_context(tc.tile_pool(name="wp", bufs=4))
        sm = ctx.enter_context(tc.tile_pool(name="sm", bufs=3))
        cn = ctx.enter_context(tc.tile_pool(name="cn", bufs=1))
        ps_mm = ctx.enter_context(tc.tile_pool(name="ps_mm", bufs=2, space="PSUM"))
        ps_sc = ctx.enter_context(tc.tile_pool(name="ps_sc", bufs=2, space="PSUM"))
        ps_ac = ctx.enter_context(tc.tile_pool(name="ps_ac", bufs=2, space="PSUM"))

        # ---- constants ----
        consts = cn.tile([P, CF], F32, tag="consts")
        nc.scalar.dma_start(consts[:], consts_d.ap())
        ones_t = cn.tile([P, 3 * P], BF16, tag="ones")
        nc.scalar.dma_start(ones_t[:],
                            wts_d.ap()[:, WOFF["ones"]:WOFF["ones"] + 3 * P])
        ones1 = ones_t[:, 0:P]           # 1.0
        onesD = ones_t[:, P:2 * P]       # 1/1024
        onesD2 = ones_t[:, 2 * P:3 * P]  # 1/512
        onesr_t = cn.tile([P, 2 * P], F32R, tag="onesr")
        nc.scalar.copy(onesr_t[:, 0:P], ones_t[:, P:2 * P])
        nc.scalar.copy(onesr_t[:, P:2 * P], ones_t[:, 2 * P:3 * P])
        onesDr = onesr_t[:, 0:P]         # 1/1024 f32r (for msq with f32r sq)
        onesD2r = onesr_t[:, P:2 * P]    # 1/512 f32r
        eps_t = cn.tile([P, 1], F32, tag="eps")
        nc.vector.memset(eps_t[:], EPS)

        def ccol(name, j):
            c0 = COFF[name] + j
            return consts[:, c0:c0 + 1]

        def cs(c):
            return slice(c * CH, (c + 1) * CH)

        # ---- dense ----
        def dense(x_sb, key, wname, bias_name, out_sb,
                  act_func=AF.Identity, residual=None, out_do=None,
                  after_c0=None):
            """out[:, do, t] = f(sum_k W x + b) (+res); weights pre-packed.

            Streams n_slab contiguous slab DMAs once, then consumes c-major
            across the whole dense (chunk 0 of every do-tile first), so a
            downstream dense needing all do-tiles of chunk c can start at the
            50% mark, and upstream producers get overlap for chunk 1.
            """
            kj_n, n_do, slab_dos = DENSES[key]
            n_slab = (n_do + slab_dos - 1) // slab_dos
            X = slab_dos * kj_n * P
            wb = WOFF[wname]
            wts = []
            for s in range(n_slab):
                wt = wp.tile([P, X], BF16, tag="w", bufs=8)
                # alternate the two hardware DGE queues for 2x fetch bandwidth
                eng = nc.sync if s % 2 == 0 else nc.scalar
                eng.dma_start(
                    wt[:], wts_d.ap()[:, wb + s * X:wb + (s + 1) * X])
                wts.append(wt)
            for c in range(NCH):
                for s in range(n_slab):
                    for g in range(slab_dos):
                        i = s * slab_dos + g
                        if i >= n_do:
                            break
                        do = out_do[i] if out_do is not None else i
                        acc = ps_mm.tile([P, CH], F32, tag="mm", bufs=2)
                        for kj in range(kj_n):
                            col = (g * kj_n + kj) * P
                            nc.tensor.matmul(
                                acc[:], wts[s][:, col:col + P],
                                x_sb[:, kj, cs(c)],
                                start=(kj == 0), stop=(kj == kj_n - 1))
                        if residual is not None:
                            nc.vector.scalar_tensor_tensor(
                                out=out_sb[:, do, cs(c)], in0=acc[:],
                                scalar=ccol(bias_name, do),
                                in1=residual[:, do, cs(c)],
                                op0=ALU.add, op1=ALU.add)
                        else:
                            nc.scalar.activation(
                                out_sb[:, do, cs(c)], acc[:], act_func,
                                bias=ccol(bias_name, i), scale=1.0)
                if c == 0 and after_c0 is not None:
                    after_c0()

        # ---- layernorm (one chunk; stats borrow the attention PSUM banks,
        # which are idle during dense phases, so chunk-0 LN emitted between a
        # producer dense's chunk passes never contends with its "mm" accs) ----
        def ln_chunk(x_sb, nj, ones_sl, onesr_sl, ng_name, b_name, out_sb, c,
                     act_func=AF.Identity):
            mean = ps_sc.tile([P, CH], F32, tag="sc", bufs=2)
            for j in range(nj):
                nc.tensor.matmul(mean[:], ones_sl, x_sb[:, j, cs(c)],
                                 start=(j == 0), stop=(j == nj - 1))
            msq = ps_ac.tile([P, CH], F32, tag="oacc")
            for j in range(nj):
                sq = sm.tile([P, CH], F32R, tag="sq", bufs=2)
                nc.scalar.square(sq[:], x_sb[:, j, cs(c)])
                nc.tensor.matmul(msq[:], onesr_sl, sq[:],
                                 start=(j == 0), stop=(j == nj - 1))
            meanS = sm.tile([P, CH], F32, tag="st", bufs=3)
            nc.scalar.copy(meanS[:], mean[:])
            m2 = sm.tile([P, CH], F32, tag="st", bufs=3)
            nc.scalar.square(m2[:], mean[:])
            var = sm.tile([P, CH], F32, tag="st", bufs=3)
            nc.vector.tensor_sub(var[:], msq[:], m2[:])
            nc.scalar.activation(var[:], var[:], AF.Sqrt, bias=eps_t[:, 0:1])
            nc.vector.reciprocal(var[:], var[:])
            for j in range(nj):
                tc_t = sm.tile([P, CH], F32, tag="tc", bufs=2)
                # tc = mean - x ; tc *= rstd ; out = act(tc*(-g)+b)
                nc.vector.scalar_tensor_tensor(
                    out=tc_t[:], in0=meanS[:], scalar=1.0,
                    in1=x_sb[:, j, cs(c)], op0=ALU.mult, op1=ALU.subtract)
                nc.vector.tensor_mul(tc_t[:], tc_t[:], var[:])
                nc.scalar.activation(
                    out_sb[:, j, cs(c)], tc_t[:], act_func,
                    bias=ccol(b_name, j), scale=ccol(ng_name, j))

        def layernorm(x_sb, nj, ones_sl, onesr_sl, ng_name, b_name, out_sb,
                      act_func=AF.Identity):
            for c in range(NCH):
                ln_chunk(x_sb, nj, ones_sl, onesr_sl, ng_name, b_name,
                         out_sb, c, act_func)

        # ================= encoder =================
        xT = act.tile([P, 6, T], BF16, tag="A")
        xT3 = xT_d.ap().rearrange("p (j t) -> p j t", j=6)
        nc.scalar.dma_start(xT[:, :, 0:CH], xT3[:, :, 0:CH])
        nc.scalar.dma_start(xT[:, :, CH:T], xT3[:, :, CH:T])
        from functools import partial
        enc_pre = act.tile([P, 8, T], BF16, tag="A")
        h = act.tile([P, 8, T], BF16, tag="A")
        dense(xT, "enc", "enc", "enc_b", enc_pre,
              after_c0=partial(ln_chunk, enc_pre, 8, onesD, onesDr,
                               "enc_ng", "enc_gb", h, 0, AF.Relu))
        ln_chunk(enc_pre, 8, onesD, onesDr, "enc_ng", "enc_gb", h, 1, AF.Relu)

        # ================= layers =================
        for l in range(L):
            qkvT = big.tile([P, 24, T], BF16, tag="B")
            dense(h, "qk", f"qk_{l}", f"qk_b_{l}", qkvT, out_do=QKV_ORDER)

            # V token-major: psum[t, dh-cols] = h[:, kj, tt]^T @ WvT
            vb0 = WOFF[f"v_{l}"]
            for hg in range(2):
                wv = wp.tile([P, 8 * CH], BF16, tag="wv", bufs=2)
                eng = nc.sync if hg == 0 else nc.scalar
                eng.dma_start(
                    wv[:],
                    wts_d.ap()[:, vb0 + hg * 8 * CH:vb0 + (hg + 1) * 8 * CH])
                for tt in range(8):
                    vp = ps_mm.tile([P, CH], F32, tag="mm", bufs=2)
                    for kj in range(8):
                        nc.tensor.matmul(
                            vp[:], h[:, kj, tt * P:(tt + 1) * P],
                            wv[:, kj * CH:(kj + 1) * CH],
                            start=(kj == 0), stop=(kj == 7))
                    nc.vector.tensor_scalar_add(
                        qkvT[:, 16 + tt, hg * CH:(hg + 1) * CH], vp[:], 0.0)

            oT = act.tile([P, 8, T], BF16, tag="A")
            for hd in range(H):
                q_sl = qkvT[:, hd, :]
                k_sl = qkvT[:, 8 + hd, :]
                for c in range(NCH):
                    o_ps = ps_ac.tile([P, CH], F32, tag="oacc")
                    se_ps = ps_ac.tile([P, CH], F32, tag="seacc")
                    # scores + exp run two tiles ahead of P@V
                    pTs = [None] * 8

                    def score_exp(j, q_sl=q_sl, k_sl=k_sl, c=c, pTs=pTs):
                        sc_ps = ps_sc.tile([P, CH], F32, tag="sc", bufs=2)
                        nc.tensor.matmul(sc_ps[:], k_sl[:, j * P:(j + 1) * P],
                                         q_sl[:, cs(c)], start=True, stop=True)
                        pT = sm.tile([P, CH], BF16, tag="pT", bufs=3)
                        nc.scalar.activation(pT[:], sc_ps[:], AF.Exp)
                        pTs[j] = pT

                    score_exp(0)
                    score_exp(1)
                    for j in range(8):
                        if j + 2 < 8:
                            score_exp(j + 2)
                        nc.tensor.matmul(o_ps[:],
                                         qkvT[:, 16 + j, hd * P:(hd + 1) * P],
                                         pTs[j][:],
                                         start=(j == 0), stop=(j == 7))
                        nc.tensor.matmul(se_ps[:], ones1, pTs[j][:],
                                         start=(j == 0), stop=(j == 7))
                    rec = sm.tile([P, CH], F32, tag="st", bufs=3)
                    nc.vector.reciprocal(rec[:], se_ps[:])
                    nc.vector.tensor_mul(oT[:, hd, cs(c)], o_ps[:], rec[:])
                    # V bias enters after normalization: o = (P@V)/sum + b_v
                    nc.vector.tensor_scalar_add(oT[:, hd, cs(c)],
                                                oT[:, hd, cs(c)],
                                                ccol(f"vb_{l}", hd))

            hn = act.tile([P, 8, T], BF16, tag="A")
            dense(oT, "out", f"out_{l}", f"out_b_{l}", h, residual=h,
                  after_c0=partial(ln_chunk, h, 8, onesD, onesDr,
                                   f"ln1_ng_{l}", f"ln1_b_{l}", hn, 0))
            ln_chunk(h, 8, onesD, onesDr, f"ln1_ng_{l}", f"ln1_b_{l}", hn, 1)

            fT = big.tile([P, 16, T], BF16, tag="B")
            dense(hn, "ff1", f"ff1_{l}", f"ff1_b_{l}", fT, act_func=AF.Relu)
            h = act.tile([P, 8, T], BF16, tag="A")
            dense(fT, "ff2", f"ff2_{l}", f"ff2_b_{l}", hn, residual=hn,
                  after_c0=partial(ln_chunk, hn, 8, onesD, onesDr,
                                   f"ln2_ng_{l}", f"ln2_b_{l}", h, 0))
            ln_chunk(hn, 8, onesD, onesDr, f"ln2_ng_{l}", f"ln2_b_{l}", h, 1)

        # ================= decoder =================
        d_pre = act.tile([P, 4, T], BF16, tag="A")
        dn = act.tile([P, 4, T], BF16, tag="A")
        dense(h, "dec1", "dec1", "dec_b1", d_pre,
              after_c0=partial(ln_chunk, d_pre, 4, onesD2, onesD2r,
                               "dec_ng", "dec_gb", dn, 0, AF.Relu))
        ln_chunk(d_pre, 4, onesD2, onesD2r, "dec_ng", "dec_gb", dn, 1,
                 AF.Relu)

        outT = big.tile([P, 8, T], F32, tag="B")
        out3 = out_d.ap().rearrange("(j p) t -> p j t", p=P)
        dense(dn, "dec2", "dec2", "dec_b2", outT,
              after_c0=lambda: nc.scalar.dma_start(
                  out3[:, :, 0:CH], outT[:, :E // P, 0:CH]))
        # per-do chunk-1 stores: each starts as soon as its do-tile is
        # written, overlapping the drain with the tail of the dense
        for do in range(E // P):
            nc.scalar.dma_start(out3[:, do:do + 1, CH:T],
                                outT[:, do:do + 1, CH:T])

    nc.compile()
    return nc


_NC_CACHE = {}


def _get_nc():
    if "nc" not in _NC_CACHE:
        _NC_CACHE["nc"] = _build()
    return _NC_CACHE["nc"]


# ------------------------------------------------------------- host packing --

def _pack_dense(W, key, order=None):
    """W: [n_do*128, kj_n*128] fp32 -> [128, n_do*kj_n*128] bf16 in SBUF order.

    Row p holds, contiguously: for slab s, for g in slab, for kj, 128 values
    W[(s*slab_dos+g)*128 + d, kj*128 + p] over d.
    """
    kj_n, n_do, slab_dos = DENSES[key]
    n_slab = (n_do + slab_dos - 1) // slab_dos
    Wt = W.reshape(n_do, P, kj_n, P).transpose(0, 3, 2, 1)  # [do, p, kj, d]
    if order is not None:
        Wt = Wt[list(order)]
    Wt = (Wt.reshape(n_slab, slab_dos, P, kj_n, P)
            .transpose(2, 0, 1, 3, 4)
            .reshape(P, n_do * kj_n * P))
    return np.ascontiguousarray(Wt.astype(BF))


def _vec_cols(vec, n):
    return np.asarray(vec, np.float32).reshape(n, P).T  # [128, n]


def _prep_inputs(inputs):
    f32 = np.float32
    consts = np.zeros((P, CF), f32)

    def put(name, cols):
        c0 = COFF[name]
        consts[:, c0:c0 + cols.shape[1]] = cols

    put("enc_b", _vec_cols(inputs["enc_b"], 8))
    put("enc_ng", _vec_cols(-np.asarray(inputs["enc_ln_g"], f32), 8))
    put("enc_gb", _vec_cols(inputs["enc_ln_b"], 8))
    put("dec_b1", _vec_cols(inputs["dec_b1"], 4))
    put("dec_ng", _vec_cols(-np.asarray(inputs["dec_ln_g"], f32), 4))
    put("dec_gb", _vec_cols(inputs["dec_ln_b"], 4))
    put("dec_b2", _vec_cols(inputs["dec_b2"], 6))

    scale = f32(1.0 / np.sqrt(DH))
    wts = np.zeros((P, WCOLS), BF)

    def putw(name, block):
        c0 = WOFF[name]
        wts[:, c0:c0 + block.shape[1]] = block

    putw("ones", np.concatenate(
        [np.ones((P, P), f32), np.full((P, P), 1.0 / D, f32),
         np.full((P, P), 2.0 / D, f32)], axis=1).astype(BF))
    putw("enc", _pack_dense(np.asarray(inputs["enc_w"], f32), "enc"))
    putw("dec1", _pack_dense(np.asarray(inputs["dec_w1"], f32), "dec1"))
    putw("dec2", _pack_dense(np.asarray(inputs["dec_w2"], f32), "dec2"))
    base = {
        "consts": consts,
    }
    for l in range(L):
        qkv_w = np.asarray(inputs["qkv_w"][l], f32)      # [3D, D]
        qkv_b = np.asarray(inputs["qkv_b"][l], f32).copy()
        Wqk = qkv_w[:2 * D].copy()
        Wqk[:D] *= scale
        qkv_b[:D] *= scale
        putw(f"qk_{l}", _pack_dense(Wqk, "qk", order=QKV_ORDER))
        # V: [hg, p, kj, d'] rows
        Wv = (qkv_w[2 * D:].reshape(2, CH, 8, P).transpose(0, 3, 2, 1)
              .reshape(2, P, 8 * CH).transpose(1, 0, 2).reshape(P, 2 * 8 * CH))
        putw(f"v_{l}", Wv.astype(BF))
        putw(f"out_{l}", _pack_dense(np.asarray(inputs["out_w"][l], f32), "out"))
        putw(f"ff1_{l}", _pack_dense(np.asarray(inputs["ff1_w"][l], f32), "ff1"))
        putw(f"ff2_{l}", _pack_dense(np.asarray(inputs["ff2_w"][l], f32), "ff2"))

        put(f"qk_b_{l}", _vec_cols(qkv_b[:2 * D], 16)[:, QKV_ORDER])
        put(f"vb_{l}", _vec_cols(qkv_b[2 * D:], 8))
        put(f"out_b_{l}", _vec_cols(inputs["out_b"][l], 8))
        put(f"ln1_ng_{l}", _vec_cols(-np.asarray(inputs["ln1_g"][l], f32), 8))
        put(f"ln1_b_{l}", _vec_cols(inputs["ln1_b"][l], 8))
        put(f"ff1_b_{l}", _vec_cols(inputs["ff1_b"][l], 16))
        put(f"ff2_b_{l}", _vec_cols(inputs["ff2_b"][l], 8))
        put(f"ln2_ng_{l}", _vec_cols(-np.asarray(inputs["ln2_g"][l], f32), 8))
        put(f"ln2_b_{l}", _vec_cols(inputs["ln2_b"][l], 8))

    base["wts"] = wts
    x = np.asarray(inputs["x"], f32)                     # (S, B, E)
    in_maps = []
    for b in range(N_CORES):
        m = dict(base)
        xb = x[:, b, :].T.reshape(6, P, T).transpose(1, 0, 2).reshape(P, 6 * T)
        m["xT"] = np.ascontiguousarray(xb.astype(BF))
        in_maps.append(m)
    return in_maps


def run(inputs, trace=False):
    nc = _get_nc()
    in_maps = _prep_inputs(inputs)
    res = run_bass_kernel_spmd(nc, in_maps, list(range(N_CORES)), trace=trace)
    out = np.empty((S, B, E), np.float32)
    for b in range(N_CORES):
        out[:, b, :] = res.results[b]["out"].T
    return out, res


def kernel(**inputs):
    out, _ = run(inputs)
    return out


def bench(inputs, iters=20, chain=1):
    """Warm-timing of the NEFF execution across the 8 cores.

    Keeps all inputs device-resident; repeated calls time only dispatch +
    on-device execution. Returns (out, per-iter ns).
    """
    import time
    import jax
    from jax.sharding import Mesh, PartitionSpec, NamedSharding
    from jax.experimental.shard_map import shard_map
    from concourse import bass2jax as b2j
    from concourse import mybir as _mybir

    nc = _get_nc()
    in_maps = _prep_inputs(inputs)
    b2j.install_neuronx_cc_hook()

    partition_name = (nc.partition_id_tensor.name
                      if nc.partition_id_tensor else None)
    in_names, out_names, out_avals, zero_outs = [], [], [], []
    for alloc in nc.m.functions[0].allocations:
        if not isinstance(alloc, _mybir.MemoryLocationSet):
            continue
        name = alloc.memorylocations[0].name
        if alloc.kind == "ExternalInput":
            if name != partition_name:
                in_names.append(name)
        elif alloc.kind == "ExternalOutput":
            np_dt = _mybir.dt.np(alloc.dtype)
            out_names.append(name)
            out_avals.append(
                jax.core.ShapedArray(tuple(alloc.tensor_shape), np_dt))
            zero_outs.append(np.zeros(alloc.tensor_shape, np_dt))

    n_params = len(in_names)
    n_outs = len(out_names)
    all_in_names = list(in_names) + list(out_names)
    if partition_name is not None:
        all_in_names.append(partition_name)

    def _body(*args):
        operands = list(args)
        if partition_name is not None:
            operands.append(b2j.partition_id_tensor())
        outs = b2j._bass_exec_p.bind(
            *operands,
            out_avals=tuple(out_avals),
            in_names=tuple(all_in_names),
            out_names=tuple(out_names),
            lowering_input_output_aliases=(),
            sim_require_finite=True,
            sim_require_nnan=True,
            nc=nc,
        )
        return tuple(outs)

    devices = jax.devices()[:N_CORES]
    mesh = Mesh(np.asarray(devices), ("core",))
    in_specs = (PartitionSpec("core"),) * (n_params + n_outs)
    out_specs = (PartitionSpec("core"),) * n_outs
    fn = jax.jit(shard_map(_body, mesh=mesh, in_specs=in_specs,
                           out_specs=out_specs, check_rep=False),
                 keep_unused=True)

    shard = NamedSharding(mesh, PartitionSpec("core"))
    concat_in = [
        jax.device_put(
            np.concatenate([np.asarray(in_maps[c][nm]) for c in range(N_CORES)],
                           axis=0), shard)
        for nm in in_names
    ]
    concat_zero = [
        jax.device_put(np.zeros((N_CORES * z.shape[0], *z.shape[1:]), z.dtype),
                       shard)
        for z in zero_outs
    ]
    outs = fn(*concat_in, *concat_zero)       # compile + warm-up
    jax.block_until_ready(outs)

    times = []
    for _ in range(iters):
        t0 = time.perf_counter()
        outs = None
        for _c in range(chain):
            outs = fn(*concat_in, *concat_zero)
        jax.block_until_ready(outs)
        times.append((time.perf_counter() - t0) * 1e9)

    out = np.empty((S, B, E), np.float32)
    oarr = np.asarray(outs[out_names.index("out")]).reshape(N_CORES, E, T)
    for b in range(N_CORES):
        out[:, b, :] = oarr[b].T
    return out, times
